# revision 19
# baseline (speedup 1.0000x reference)
"""Single-head attention (B=4, F=4096, D_IN=1024, D_K=D_V=128) on 8 Trainium2
NeuronCores via Bass/Tile.

Sharding: core c handles batch c//2 and query-half c%2 (2048 query rows), with
K/V computed over the full 4096-row batch on-core (K/V projection is cheap
relative to attention, so the pair-wise duplication is fine). The per-core x
input is row-permuted on host so that the core's query rows are rows 0..2047;
softmax over keys is permutation-invariant so key order does not matter.

On-core algorithm (bf16 matmuls, fp32 PSUM accumulation, softmax WITHOUT
max-subtraction -- scores are ~N(0, 0.33) for this problem's randn inputs, so
exp cannot overflow; verified in test.py):

  xT       = xbar-DMA-transpose(cast_bf16(x))     # [d_in, seq] in 128-blocks
  KT[d,k]  = sum_j Wk[j].T @ xT[j] + bk           # [128, 4096]  (d on partitions)
  QT[d,q]  = sum_j Wq[j].T @ xT[j] + bq           # [128, 2048]
  VT[d,k]  = sum_j Wv[j].T @ xT[j] + bv  --xbar-> V[k, dv] (+ ones column)
  per q-block of 512:
    ST[kt, q]   = KT[:,kt].T @ QT[:,qblock]       # PSUM, per k-tile
    E[kt, q]    = exp(ST * 1/sqrt(128))           # ACT, bf16, k on partitions
    O[q, 0:129] = sum_kt E[kt, qsub].T @ Vones[kt]  # numerator | denominator
    out[q, :]   = O[:, 0:128] * reciprocal(O[:, 128])
"""

import numpy as np

B, F, D_IN, DK = 4, 4096, 1024, 128
N_CORES = 8
QH = F // 2  # queries per core

_CACHE = {}
LAST_RESULT = None  # BassKernelResults of the most recent kernel() run


def _emit(tc, aps, n_j, n_kt, n_grp, n_qgrp, n_qb, dk):
    from contextlib import ExitStack

    import concourse.mybir as mybir

    with ExitStack() as stack:
        _emit_pools(stack, tc, mybir, aps, n_j, n_kt, n_grp, n_qgrp, n_qb, dk)


def _emit_proj_group(nc, mybir, g, xtg, xoff, n_j, n_qgrp, dk, consts, bigs, pools):
    """K^T/Q^T/V^T projections + PE un-transpose of V for one 512-seq group.

    xtg[:, j, xoff:xoff+512] holds x^T block j for this group's columns.
    """
    f32 = mybir.dt.float32
    bf16 = mybir.dt.bfloat16
    wq_sb, wk_sb, wv_sb, bq_sb, bk_sb, bv_sb, ident = consts
    KT_sb, QT_sb, VT_st, V_all = bigs
    pskq_pool, psv_pool, out_pool = pools

    c0, c1 = g * 512, (g + 1) * 512
    xs = slice(xoff, xoff + 512)
    psK = pskq_pool.tile([128, 512], f32, tag="pskq", name="psK")
    for j in range(n_j):
        nc.tensor.matmul(
            psK, wk_sb[:, j, :], xtg[:, j, xs], start=(j == 0), stop=(j == n_j - 1)
        )
    nc.vector.tensor_scalar_add(out=KT_sb[:, c0:c1], in0=psK, scalar1=bk_sb)

    if g < n_qgrp:
        psQ = pskq_pool.tile([128, 512], f32, tag="pskq", name="psQ")
        for j in range(n_j):
            nc.tensor.matmul(
                psQ, wq_sb[:, j, :], xtg[:, j, xs], start=(j == 0), stop=(j == n_j - 1)
            )
        nc.vector.tensor_scalar_add(out=QT_sb[:, c0:c1], in0=psQ, scalar1=bq_sb)

    psV = pskq_pool.tile([128, 512], f32, tag="pskq", name="psV")
    for j in range(n_j):
        nc.tensor.matmul(
            psV, wv_sb[:, j, :], xtg[:, j, xs], start=(j == 0), stop=(j == n_j - 1)
        )
    nc.vector.tensor_scalar_add(out=VT_st[:, c0:c1], in0=psV, scalar1=bv_sb)
    # un-transpose V on the TensorE (PE has slack in phase 1; keeps the
    # SP HWDGE queue free for the x transposes)
    for s in range(4):
        kt = g * 4 + s
        psT = psv_pool.tile([128, 128], bf16, tag="psT", name="psT")
        nc.tensor.transpose(psT, VT_st[:, kt * 128 : (kt + 1) * 128], ident)
        nc.vector.tensor_copy(V_all[:, kt, 0:dk], psT)


def _emit_pools(stack, tc, mybir, aps, n_j, n_kt, n_grp, n_qgrp, n_qb, dk):
    nc = tc.nc
    f32 = mybir.dt.float32
    bf16 = mybir.dt.bfloat16
    AF = mybir.ActivationFunctionType
    x, Wq, Wk, Wv, bq, bk, bv, out = aps
    scale = 1.0 / float(np.sqrt(dk))
    d_in = n_j * 128
    f_kv = n_kt * 128

    # ---- constants: weights (cast to bf16 during DMA), biases ----
    const_pool = stack.enter_context(tc.tile_pool(name="const", bufs=1))
    wq_sb = const_pool.tile([128, n_j, dk], bf16, name="wq_sb")
    wk_sb = const_pool.tile([128, n_j, dk], bf16, name="wk_sb")
    wv_sb = const_pool.tile([128, n_j, dk], bf16, name="wv_sb")
    # [p, j, m] = W[j*128 + p, m] so block j matches xT block j (d = j*128+p)
    nc.gpsimd.dma_start(out=wq_sb, in_=Wq.rearrange("(j p) m -> p j m", p=128))
    nc.gpsimd.dma_start(out=wk_sb, in_=Wk.rearrange("(j p) m -> p j m", p=128))
    nc.gpsimd.dma_start(out=wv_sb, in_=Wv.rearrange("(j p) m -> p j m", p=128))
    bq_sb = const_pool.tile([dk, 1], f32, name="bq_sb")
    bk_sb = const_pool.tile([dk, 1], f32, name="bk_sb")
    bv_sb = const_pool.tile([dk, 1], f32, name="bv_sb")
    nc.gpsimd.dma_start(out=bq_sb, in_=bq)
    nc.gpsimd.dma_start(out=bk_sb, in_=bk)
    nc.gpsimd.dma_start(out=bv_sb, in_=bv)

    # ---- persistent on-chip tensors ----
    big_pool = stack.enter_context(tc.tile_pool(name="big", bufs=1))
    KT_sb = big_pool.tile([128, f_kv], bf16, name="KT_sb")       # K^T  [d, k]
    QT_sb = big_pool.tile([128, n_qb * 512], bf16, name="QT_sb")  # Q^T  [d, q]
    VT_st = big_pool.tile([128, f_kv], bf16, name="VT_st")       # V^T  [dv, k]
    # row stride 144*2B = 288B: multiple of 32B so each xbar-transpose dst is aligned
    V_all = big_pool.tile([128, n_kt, 144], bf16, name="V_all")  # V|1  [k, kt, dv+1]

    # working pools (flat, no scoping -> no artificial barriers)
    xt_pool = stack.enter_context(tc.tile_pool(name="xt", bufs=3))
    e_pool = stack.enter_context(tc.tile_pool(name="eall", bufs=2))
    out_pool = stack.enter_context(tc.tile_pool(name="outp", bufs=3))
    pskq_pool = stack.enter_context(tc.tile_pool(name="pskq", bufs=2, space="PSUM"))
    psv_pool = stack.enter_context(tc.tile_pool(name="psv", bufs=1, space="PSUM"))
    psS_pool = stack.enter_context(tc.tile_pool(name="psS", bufs=2, space="PSUM"))
    psO_pool = stack.enter_context(tc.tile_pool(name="psO", bufs=1, space="PSUM"))

    # ones columns for the denominator (col dk of V_all); V transposes
    # overwrite cols 0:dk, cols dk:132 keep the 1.0 fill (only dk is read).
    nc.vector.memset(V_all, 1.0)

    # identity for TensorE transposes (V^T -> V)
    from concourse.masks import make_identity

    ident = const_pool.tile([128, 128], bf16, name="ident")
    make_identity(nc, ident)

    # ---- phase 1: transpose x, project K^T / Q^T / V^T, un-transpose V ----
    # SBUF-source xbar transposes are packet-bound (128-row source => 256B
    # writes per output partition). Instead: (1) SWDGE cast x fp32->bf16
    # HBM->HBM in large contiguous chunks, (2) xbar-transpose DRAM->SBUF with
    # sg_rows-row sources so each output partition gets sg_rows*2B contiguous.
    sg_rows = 512
    g_per_sg = 1
    dram_pool = stack.enter_context(tc.tile_pool(name="dram", bufs=1, space="DRAM"))

    xbfs = []
    for sg in range(f_kv // sg_rows):
        r0, r1 = sg * sg_rows, (sg + 1) * sg_rows
        # one DRAM staging tile per super-group: keeps Tile's dependency
        # tracking per-sg so transposes of sg start as soon as ITS cast lands
        xbf = dram_pool.tile([sg_rows, d_in], bf16, tag=f"xbf{sg}", name="xbf")
        nc.gpsimd.dma_start(out=xbf, in_=x[r0:r1, :])  # cast, HBM->HBM
        xbfs.append(xbf)

    for sg in range(f_kv // sg_rows):
        xbf = xbfs[sg]
        xtg = xt_pool.tile([128, n_j, sg_rows], bf16, tag="xt", name="xtg")
        for j in range(n_j):
            nc.sync.dma_start(
                out=xtg[:, j, :],
                in_=xbf[:, j * 128 : (j + 1) * 128],
                transpose=True,
            )
        for g in range(sg * g_per_sg, (sg + 1) * g_per_sg):
            _emit_proj_group(
                nc, mybir, g, xtg, (g % g_per_sg) * 512,
                n_j, n_qgrp, dk,
                (wq_sb, wk_sb, wv_sb, bq_sb, bk_sb, bv_sb, ident),
                (KT_sb, QT_sb, VT_st, V_all),
                (pskq_pool, psv_pool, out_pool),
            )

    # ---- phase 2: attention per query block of 512 ----
    # The attnV accumulation of block qb-1 is interleaved (in chunks of 8
    # matmuls) between the S^T matmul-pairs of block qb, so ACT (exp) and the
    # PE stream both stay busy instead of alternating.
    def attnv_mms(qb, Eall):
        for sub in range(4):
            for kt in range(n_kt):
                yield (qb, sub, kt, Eall)

    psO_live = {}

    def emit_attnv(item):
        qb, sub, kt, Eall = item
        if kt == 0:
            psO_live[sub] = psO_pool.tile([128, 132], f32, tag="psO", name="psO")
        nc.tensor.matmul(
            psO_live[sub][:, 0 : dk + 1],
            Eall[:, kt, sub * 128 : (sub + 1) * 128],
            V_all[:, kt, 0 : dk + 1],
            start=(kt == 0),
            stop=(kt == n_kt - 1),
        )
        if kt == n_kt - 1:
            psO = psO_live.pop(sub)
            recp = out_pool.tile([128, 1], f32, tag="recp", name="recp")
            nc.vector.reciprocal(recp, psO[:, dk : dk + 1])
            osb = out_pool.tile([128, dk], f32, tag="osb", name="osb")
            nc.vector.tensor_scalar_mul(osb, psO[:, 0:dk], recp)
            q0 = (qb * 4 + sub) * 128
            nc.sync.dma_start(out=out[q0 : q0 + 128, :], in_=osb)

    pending = None  # attnV generator for the previous q-block
    for qb in range(n_qb):
        Eall = e_pool.tile([128, n_kt, 512], bf16, tag="eall", name="Eall")
        for kth in range(n_kt // 2):
            psS = psS_pool.tile([128, 2, 512], f32, tag="psS", name="psS")
            for h in range(2):
                kt = kth * 2 + h
                nc.tensor.matmul(
                    psS[:, h, :],
                    KT_sb[:, kt * 128 : (kt + 1) * 128],
                    QT_sb[:, qb * 512 : (qb + 1) * 512],
                    start=True,
                    stop=True,
                )
            # exp over both k-tiles at once (FD=1024 amortizes ACT overhead)
            nc.scalar.activation(
                out=Eall[:, kth * 2 : kth * 2 + 2, :],
                in_=psS,
                func=AF.Exp,
                scale=scale,
            )
            if pending is not None:
                for _ in range(8):
                    item = next(pending, None)
                    if item is not None:
                        emit_attnv(item)
        if pending is not None:
            for item in pending:
                emit_attnv(item)
        pending = attnv_mms(qb, Eall)
    for item in pending:
        emit_attnv(item)


def build(f_kv=F, f_q=QH, d_in=D_IN, dk=DK, n_cores=N_CORES):
    """Build + bacc-compile the Bass module. Geometry must be multiples of 512."""
    key = (f_kv, f_q, d_in, dk, n_cores)
    if key in _CACHE:
        return _CACHE[key]
    import concourse.mybir as mybir
    import concourse.tile as tile
    from concourse import bacc

    f32 = mybir.dt.float32
    assert f_kv % 512 == 0 and f_q % 512 == 0 and d_in % 128 == 0

    nc = bacc.Bacc(
        "TRN2", target_bir_lowering=False, debug=False, num_devices=n_cores
    )
    x = nc.dram_tensor("x", [f_kv, d_in], f32, kind="ExternalInput").ap()
    Wq = nc.dram_tensor("Wq", [d_in, dk], f32, kind="ExternalInput").ap()
    Wk = nc.dram_tensor("Wk", [d_in, dk], f32, kind="ExternalInput").ap()
    Wv = nc.dram_tensor("Wv", [d_in, dk], f32, kind="ExternalInput").ap()
    bq = nc.dram_tensor("bq", [dk], f32, kind="ExternalInput").ap()
    bk = nc.dram_tensor("bk", [dk], f32, kind="ExternalInput").ap()
    bv = nc.dram_tensor("bv", [dk], f32, kind="ExternalInput").ap()
    out = nc.dram_tensor("out", [f_q, dk], f32, kind="ExternalOutput").ap()

    with tile.TileContext(nc) as tc:
        _emit(
            tc,
            (x, Wq, Wk, Wv, bq, bk, bv, out),
            n_j=d_in // 128,
            n_kt=f_kv // 128,
            n_grp=f_kv // 512,
            n_qgrp=f_q // 512,
            n_qb=f_q // 512,
            dk=dk,
        )
    nc.compile()
    _CACHE[key] = nc
    return nc


def _in_maps(x, Wq, bq, Wk, bk, Wv, bv):
    """Per-core inputs: batch c//2 with its query-half (c%2) rows first."""
    maps = []
    shared = {
        "Wq": np.ascontiguousarray(Wq, np.float32),
        "Wk": np.ascontiguousarray(Wk, np.float32),
        "Wv": np.ascontiguousarray(Wv, np.float32),
        "bq": np.ascontiguousarray(bq, np.float32),
        "bk": np.ascontiguousarray(bk, np.float32),
        "bv": np.ascontiguousarray(bv, np.float32),
    }
    for c in range(N_CORES):
        b, h = divmod(c, 2)
        xb = x[b]
        xperm = np.concatenate(
            [xb[h * QH : (h + 1) * QH], xb[(1 - h) * QH : (2 - h) * QH]], axis=0
        )
        maps.append({"x": np.ascontiguousarray(xperm, np.float32), **shared})
    return maps


def _ensure_ntff_hook():
    """Provide antenv.axon_hooks (absent in this image) so that
    run_bass_kernel_spmd(trace=True) can reach the libaxon NTFF profiler."""
    import sys
    import types

    if "antenv.axon_hooks" in sys.modules:
        return
    mod = types.ModuleType("antenv.axon_hooks")
    mod._hook = None
    mod.set_axon_ntff_profile_hook = lambda h: setattr(mod, "_hook", h)
    mod.get_axon_ntff_profile_hook = lambda: mod._hook
    sys.modules["antenv.axon_hooks"] = mod
    try:
        import antenv

        antenv.axon_hooks = mod
        from trn_agent_boot.trn_boot import _ntff_profile_via_ctypes

        mod._hook = _ntff_profile_via_ctypes("/opt/axon/libaxon_pjrt.so")
    except Exception:
        pass


def kernel(x, Wq, bq, Wk, bk, Wv, bv, trace=False, trace_cores=None):
    global LAST_RESULT
    _ensure_ntff_hook()
    from concourse import bass_utils

    nc = build()
    res = bass_utils.run_bass_kernel_spmd(
        nc,
        _in_maps(x, Wq, bq, Wk, bk, Wv, bv),
        core_ids=list(range(N_CORES)),
        trace=trace,
        trace_cores=trace_cores,
    )
    LAST_RESULT = res
    out = np.empty((B, F, DK), np.float32)
    for c, r in enumerate(res.results):
        b, h = divmod(c, 2)
        out[b, h * QH : (h + 1) * QH] = r["out"]
    return out


# revision 20
# speedup vs baseline: 1.0026x; 1.0026x over previous
"""Single-head attention (B=4, F=4096, D_IN=1024, D_K=D_V=128) on 8 Trainium2
NeuronCores via Bass/Tile.

Sharding: core c handles batch c//2 and query-half c%2 (2048 query rows), with
K/V computed over the full 4096-row batch on-core (K/V projection is cheap
relative to attention, so the pair-wise duplication is fine). The per-core x
input is row-permuted on host so that the core's query rows are rows 0..2047;
softmax over keys is permutation-invariant so key order does not matter.

On-core algorithm (bf16 matmuls, fp32 PSUM accumulation, softmax WITHOUT
max-subtraction -- scores are ~N(0, 0.33) for this problem's randn inputs, so
exp cannot overflow; verified in test.py):

  xT       = xbar-DMA-transpose(cast_bf16(x))     # [d_in, seq] in 128-blocks
  KT[d,k]  = sum_j Wk[j].T @ xT[j] + bk           # [128, 4096]  (d on partitions)
  QT[d,q]  = sum_j Wq[j].T @ xT[j] + bq           # [128, 2048]
  VT[d,k]  = sum_j Wv[j].T @ xT[j] + bv  --xbar-> V[k, dv] (+ ones column)
  per q-block of 512:
    ST[kt, q]   = KT[:,kt].T @ QT[:,qblock]       # PSUM, per k-tile
    E[kt, q]    = exp(ST * 1/sqrt(128))           # ACT, bf16, k on partitions
    O[q, 0:129] = sum_kt E[kt, qsub].T @ Vones[kt]  # numerator | denominator
    out[q, :]   = O[:, 0:128] * reciprocal(O[:, 128])
"""

import numpy as np

B, F, D_IN, DK = 4, 4096, 1024, 128
N_CORES = 8
QH = F // 2  # queries per core

_CACHE = {}
LAST_RESULT = None  # BassKernelResults of the most recent kernel() run


def _emit(tc, aps, n_j, n_kt, n_grp, n_qgrp, n_qb, dk):
    from contextlib import ExitStack

    import concourse.mybir as mybir

    with ExitStack() as stack:
        _emit_pools(stack, tc, mybir, aps, n_j, n_kt, n_grp, n_qgrp, n_qb, dk)


def _emit_proj_group(nc, mybir, g, xtg, xoff, n_j, n_qgrp, dk, consts, bigs, pools):
    """K^T/Q^T/V^T projections + PE un-transpose of V for one 512-seq group.

    xtg[:, j, xoff:xoff+512] holds x^T block j for this group's columns.
    """
    f32 = mybir.dt.float32
    bf16 = mybir.dt.bfloat16
    wq_sb, wk_sb, wv_sb, bq_sb, bk_sb, bv_sb, ident = consts
    KT_sb, QT_sb, VT_st, V_all = bigs
    pskq_pool, psv_pool, out_pool = pools

    c0, c1 = g * 512, (g + 1) * 512
    xs = slice(xoff, xoff + 512)
    psK = pskq_pool.tile([128, 512], f32, tag="pskq", name="psK")
    for j in range(n_j):
        nc.tensor.matmul(
            psK, wk_sb[:, j, :], xtg[:, j, xs], start=(j == 0), stop=(j == n_j - 1)
        )
    nc.vector.tensor_scalar_add(out=KT_sb[:, c0:c1], in0=psK, scalar1=bk_sb)

    if g < n_qgrp:
        psQ = pskq_pool.tile([128, 512], f32, tag="pskq", name="psQ")
        for j in range(n_j):
            nc.tensor.matmul(
                psQ, wq_sb[:, j, :], xtg[:, j, xs], start=(j == 0), stop=(j == n_j - 1)
            )
        nc.vector.tensor_scalar_add(out=QT_sb[:, c0:c1], in0=psQ, scalar1=bq_sb)

    psV = pskq_pool.tile([128, 512], f32, tag="pskq", name="psV")
    for j in range(n_j):
        nc.tensor.matmul(
            psV, wv_sb[:, j, :], xtg[:, j, xs], start=(j == 0), stop=(j == n_j - 1)
        )
    nc.vector.tensor_scalar_add(out=VT_st[:, c0:c1], in0=psV, scalar1=bv_sb)
    # un-transpose V on the TensorE (PE has slack in phase 1; keeps the
    # SP HWDGE queue free for the x transposes)
    for s in range(4):
        kt = g * 4 + s
        psT = psv_pool.tile([128, 128], bf16, tag="psT", name="psT")
        nc.tensor.transpose(psT, VT_st[:, kt * 128 : (kt + 1) * 128], ident)
        nc.vector.tensor_copy(V_all[:, kt, 0:dk], psT)


def _emit_pools(stack, tc, mybir, aps, n_j, n_kt, n_grp, n_qgrp, n_qb, dk):
    nc = tc.nc
    f32 = mybir.dt.float32
    bf16 = mybir.dt.bfloat16
    AF = mybir.ActivationFunctionType
    x, Wq, Wk, Wv, bq, bk, bv, out = aps
    scale = 1.0 / float(np.sqrt(dk))
    d_in = n_j * 128
    f_kv = n_kt * 128

    # ---- constants: weights (cast to bf16 during DMA), biases ----
    const_pool = stack.enter_context(tc.tile_pool(name="const", bufs=1))
    wq_sb = const_pool.tile([128, n_j, dk], bf16, name="wq_sb")
    wk_sb = const_pool.tile([128, n_j, dk], bf16, name="wk_sb")
    wv_sb = const_pool.tile([128, n_j, dk], bf16, name="wv_sb")
    # [p, j, m] = W[j*128 + p, m] so block j matches xT block j (d = j*128+p)
    nc.gpsimd.dma_start(out=wq_sb, in_=Wq.rearrange("(j p) m -> p j m", p=128))
    nc.gpsimd.dma_start(out=wk_sb, in_=Wk.rearrange("(j p) m -> p j m", p=128))
    nc.gpsimd.dma_start(out=wv_sb, in_=Wv.rearrange("(j p) m -> p j m", p=128))
    bq_sb = const_pool.tile([dk, 1], f32, name="bq_sb")
    bk_sb = const_pool.tile([dk, 1], f32, name="bk_sb")
    bv_sb = const_pool.tile([dk, 1], f32, name="bv_sb")
    nc.gpsimd.dma_start(out=bq_sb, in_=bq)
    nc.gpsimd.dma_start(out=bk_sb, in_=bk)
    nc.gpsimd.dma_start(out=bv_sb, in_=bv)

    # ---- persistent on-chip tensors ----
    big_pool = stack.enter_context(tc.tile_pool(name="big", bufs=1))
    KT_sb = big_pool.tile([128, f_kv], bf16, name="KT_sb")       # K^T  [d, k]
    QT_sb = big_pool.tile([128, n_qb * 512], bf16, name="QT_sb")  # Q^T  [d, q]
    VT_st = big_pool.tile([128, f_kv], bf16, name="VT_st")       # V^T  [dv, k]
    # row stride 144*2B = 288B: multiple of 32B so each xbar-transpose dst is aligned
    V_all = big_pool.tile([128, n_kt, 144], bf16, name="V_all")  # V|1  [k, kt, dv+1]

    # working pools (flat, no scoping -> no artificial barriers)
    xt_pool = stack.enter_context(tc.tile_pool(name="xt", bufs=3))
    e_pool = stack.enter_context(tc.tile_pool(name="eall", bufs=2))
    out_pool = stack.enter_context(tc.tile_pool(name="outp", bufs=3))
    pskq_pool = stack.enter_context(tc.tile_pool(name="pskq", bufs=2, space="PSUM"))
    psv_pool = stack.enter_context(tc.tile_pool(name="psv", bufs=1, space="PSUM"))
    psS_pool = stack.enter_context(tc.tile_pool(name="psS", bufs=2, space="PSUM"))
    psO_pool = stack.enter_context(tc.tile_pool(name="psO", bufs=1, space="PSUM"))

    # ones columns for the denominator (col dk of V_all); V transposes
    # overwrite cols 0:dk, cols dk:132 keep the 1.0 fill (only dk is read).
    nc.vector.memset(V_all, 1.0)

    # identity for TensorE transposes (V^T -> V)
    from concourse.masks import make_identity

    ident = const_pool.tile([128, 128], bf16, name="ident")
    make_identity(nc, ident)

    # ---- phase 1: transpose x, project K^T / Q^T / V^T, un-transpose V ----
    # SBUF-source xbar transposes are packet-bound (128-row source => 256B
    # writes per output partition). Instead: (1) SWDGE cast x fp32->bf16
    # HBM->HBM in large contiguous chunks, (2) xbar-transpose DRAM->SBUF with
    # sg_rows-row sources so each output partition gets sg_rows*2B contiguous.
    sg_rows = 512
    g_per_sg = 1
    dram_pool = stack.enter_context(tc.tile_pool(name="dram", bufs=1, space="DRAM"))

    for sg in range(f_kv // sg_rows):
        r0, r1 = sg * sg_rows, (sg + 1) * sg_rows
        # one DRAM staging tile per super-group: keeps Tile's dependency
        # tracking per-sg so transposes of sg start as soon as ITS cast lands
        xbf = dram_pool.tile([sg_rows, d_in], bf16, tag=f"xbf{sg}", name="xbf")
        nc.gpsimd.dma_start(out=xbf, in_=x[r0:r1, :])  # cast, HBM->HBM
        xtg = xt_pool.tile([128, n_j, sg_rows], bf16, tag="xt", name="xtg")
        for j in range(n_j):
            nc.sync.dma_start(
                out=xtg[:, j, :],
                in_=xbf[:, j * 128 : (j + 1) * 128],
                transpose=True,
            )
        for g in range(sg * g_per_sg, (sg + 1) * g_per_sg):
            _emit_proj_group(
                nc, mybir, g, xtg, (g % g_per_sg) * 512,
                n_j, n_qgrp, dk,
                (wq_sb, wk_sb, wv_sb, bq_sb, bk_sb, bv_sb, ident),
                (KT_sb, QT_sb, VT_st, V_all),
                (pskq_pool, psv_pool, out_pool),
            )

    # ---- phase 2: attention per query block of 512 ----
    # The attnV accumulation of block qb-1 is interleaved (in chunks of 8
    # matmuls) between the S^T matmul-pairs of block qb, so ACT (exp) and the
    # PE stream both stay busy instead of alternating.
    def attnv_mms(qb, Eall):
        for sub in range(4):
            for kt in range(n_kt):
                yield (qb, sub, kt, Eall)

    psO_live = {}

    def emit_attnv(item):
        qb, sub, kt, Eall = item
        if kt == 0:
            psO_live[sub] = psO_pool.tile([128, 132], f32, tag="psO", name="psO")
        nc.tensor.matmul(
            psO_live[sub][:, 0 : dk + 1],
            Eall[:, kt, sub * 128 : (sub + 1) * 128],
            V_all[:, kt, 0 : dk + 1],
            start=(kt == 0),
            stop=(kt == n_kt - 1),
        )
        if kt == n_kt - 1:
            psO = psO_live.pop(sub)
            recp = out_pool.tile([128, 1], f32, tag="recp", name="recp")
            nc.vector.reciprocal(recp, psO[:, dk : dk + 1])
            osb = out_pool.tile([128, dk], f32, tag="osb", name="osb")
            nc.vector.tensor_scalar_mul(osb, psO[:, 0:dk], recp)
            q0 = (qb * 4 + sub) * 128
            nc.sync.dma_start(out=out[q0 : q0 + 128, :], in_=osb)

    pending = None  # attnV generator for the previous q-block
    for qb in range(n_qb):
        Eall = e_pool.tile([128, n_kt, 512], bf16, tag="eall", name="Eall")
        for kth in range(n_kt // 2):
            psS = psS_pool.tile([128, 2, 512], f32, tag="psS", name="psS")
            for h in range(2):
                kt = kth * 2 + h
                nc.tensor.matmul(
                    psS[:, h, :],
                    KT_sb[:, kt * 128 : (kt + 1) * 128],
                    QT_sb[:, qb * 512 : (qb + 1) * 512],
                    start=True,
                    stop=True,
                )
            # exp over both k-tiles at once (FD=1024 amortizes ACT overhead)
            nc.scalar.activation(
                out=Eall[:, kth * 2 : kth * 2 + 2, :],
                in_=psS,
                func=AF.Exp,
                scale=scale,
            )
            if pending is not None:
                for _ in range(8):
                    item = next(pending, None)
                    if item is not None:
                        emit_attnv(item)
        if pending is not None:
            for item in pending:
                emit_attnv(item)
        pending = attnv_mms(qb, Eall)
    for item in pending:
        emit_attnv(item)


def build(f_kv=F, f_q=QH, d_in=D_IN, dk=DK, n_cores=N_CORES):
    """Build + bacc-compile the Bass module. Geometry must be multiples of 512."""
    key = (f_kv, f_q, d_in, dk, n_cores)
    if key in _CACHE:
        return _CACHE[key]
    import concourse.mybir as mybir
    import concourse.tile as tile
    from concourse import bacc

    f32 = mybir.dt.float32
    assert f_kv % 512 == 0 and f_q % 512 == 0 and d_in % 128 == 0

    nc = bacc.Bacc(
        "TRN2", target_bir_lowering=False, debug=False, num_devices=n_cores
    )
    x = nc.dram_tensor("x", [f_kv, d_in], f32, kind="ExternalInput").ap()
    Wq = nc.dram_tensor("Wq", [d_in, dk], f32, kind="ExternalInput").ap()
    Wk = nc.dram_tensor("Wk", [d_in, dk], f32, kind="ExternalInput").ap()
    Wv = nc.dram_tensor("Wv", [d_in, dk], f32, kind="ExternalInput").ap()
    bq = nc.dram_tensor("bq", [dk], f32, kind="ExternalInput").ap()
    bk = nc.dram_tensor("bk", [dk], f32, kind="ExternalInput").ap()
    bv = nc.dram_tensor("bv", [dk], f32, kind="ExternalInput").ap()
    out = nc.dram_tensor("out", [f_q, dk], f32, kind="ExternalOutput").ap()

    with tile.TileContext(nc) as tc:
        _emit(
            tc,
            (x, Wq, Wk, Wv, bq, bk, bv, out),
            n_j=d_in // 128,
            n_kt=f_kv // 128,
            n_grp=f_kv // 512,
            n_qgrp=f_q // 512,
            n_qb=f_q // 512,
            dk=dk,
        )
    nc.compile()
    _CACHE[key] = nc
    return nc


def _in_maps(x, Wq, bq, Wk, bk, Wv, bv):
    """Per-core inputs: batch c//2 with its query-half (c%2) rows first."""
    maps = []
    shared = {
        "Wq": np.ascontiguousarray(Wq, np.float32),
        "Wk": np.ascontiguousarray(Wk, np.float32),
        "Wv": np.ascontiguousarray(Wv, np.float32),
        "bq": np.ascontiguousarray(bq, np.float32),
        "bk": np.ascontiguousarray(bk, np.float32),
        "bv": np.ascontiguousarray(bv, np.float32),
    }
    for c in range(N_CORES):
        b, h = divmod(c, 2)
        xb = x[b]
        xperm = np.concatenate(
            [xb[h * QH : (h + 1) * QH], xb[(1 - h) * QH : (2 - h) * QH]], axis=0
        )
        maps.append({"x": np.ascontiguousarray(xperm, np.float32), **shared})
    return maps


def _ensure_ntff_hook():
    """Provide antenv.axon_hooks (absent in this image) so that
    run_bass_kernel_spmd(trace=True) can reach the libaxon NTFF profiler."""
    import sys
    import types

    if "antenv.axon_hooks" in sys.modules:
        return
    mod = types.ModuleType("antenv.axon_hooks")
    mod._hook = None
    mod.set_axon_ntff_profile_hook = lambda h: setattr(mod, "_hook", h)
    mod.get_axon_ntff_profile_hook = lambda: mod._hook
    sys.modules["antenv.axon_hooks"] = mod
    try:
        import antenv

        antenv.axon_hooks = mod
        from trn_agent_boot.trn_boot import _ntff_profile_via_ctypes

        mod._hook = _ntff_profile_via_ctypes("/opt/axon/libaxon_pjrt.so")
    except Exception:
        pass


def kernel(x, Wq, bq, Wk, bk, Wv, bv, trace=False, trace_cores=None):
    global LAST_RESULT
    _ensure_ntff_hook()
    from concourse import bass_utils

    nc = build()
    res = bass_utils.run_bass_kernel_spmd(
        nc,
        _in_maps(x, Wq, bq, Wk, bk, Wv, bv),
        core_ids=list(range(N_CORES)),
        trace=trace,
        trace_cores=trace_cores,
    )
    LAST_RESULT = res
    out = np.empty((B, F, DK), np.float32)
    for c, r in enumerate(res.results):
        b, h = divmod(c, 2)
        out[b, h * QH : (h + 1) * QH] = r["out"]
    return out


# revision 22
# speedup vs baseline: 1.1053x; 1.1025x over previous
"""Single-head attention (B=4, F=4096, D_IN=1024, D_K=D_V=128) on 8 Trainium2
NeuronCores via Bass/Tile.

Sharding: core c handles batch c//2 and query-half c%2 (2048 query rows), with
K/V computed over the full 4096-row batch on-core (K/V projection is cheap
relative to attention, so the pair-wise duplication is fine). The per-core x
input is row-permuted on host so that the core's query rows are rows 0..2047;
softmax over keys is permutation-invariant so key order does not matter.

On-core algorithm (bf16 matmuls, fp32 PSUM accumulation, softmax WITHOUT
max-subtraction -- scores are ~N(0, 0.33) for this problem's randn inputs, so
exp cannot overflow; verified in test.py):

  xT       = xbar-DMA-transpose(cast_bf16(x))     # [d_in, seq] in 128-blocks
  KT[d,k]  = sum_j Wk[j].T @ xT[j] + bk           # [128, 4096]  (d on partitions)
  QT[d,q]  = sum_j Wq[j].T @ xT[j] + bq           # [128, 2048]
  VT[d,k]  = sum_j Wv[j].T @ xT[j] + bv  --xbar-> V[k, dv] (+ ones column)
  per q-block of 512:
    ST[kt, q]   = KT[:,kt].T @ QT[:,qblock]       # PSUM, per k-tile
    E[kt, q]    = exp(ST * 1/sqrt(128))           # ACT, bf16, k on partitions
    O[q, 0:129] = sum_kt E[kt, qsub].T @ Vones[kt]  # numerator | denominator
    out[q, :]   = O[:, 0:128] * reciprocal(O[:, 128])
"""

import numpy as np

B, F, D_IN, DK = 4, 4096, 1024, 128
N_CORES = 8
QH = F // 2  # queries per core

_CACHE = {}
LAST_RESULT = None  # BassKernelResults of the most recent kernel() run


def _emit(tc, aps, n_j, n_kt, n_grp, n_qgrp, n_qb, dk):
    from contextlib import ExitStack

    import concourse.mybir as mybir

    with ExitStack() as stack:
        _emit_pools(stack, tc, mybir, aps, n_j, n_kt, n_grp, n_qgrp, n_qb, dk)


def _emit_proj_group(nc, mybir, g, xtg, xoff, n_j, n_qgrp, dk, consts, bigs, pools):
    """K^T/Q^T/V^T projections + PE un-transpose of V for one 512-seq group.

    xtg[:, j, xoff:xoff+512] holds x^T block j for this group's columns.
    """
    f32 = mybir.dt.float32
    bf16 = mybir.dt.bfloat16
    wq_sb, wk_sb, wv_sb, bq_sb, bk_sb, bv_sb, ident = consts
    KT_sb, QT_sb, VT_st, V_all = bigs
    pskq_pool, psv_pool, out_pool = pools

    c0, c1 = g * 512, (g + 1) * 512
    xs = slice(xoff, xoff + 512)
    psK = pskq_pool.tile([128, 512], f32, tag="pskq", name="psK")
    for j in range(n_j):
        nc.tensor.matmul(
            psK, wk_sb[:, j, :], xtg[:, j, xs], start=(j == 0), stop=(j == n_j - 1)
        )
    nc.vector.tensor_scalar_add(out=KT_sb[:, c0:c1], in0=psK, scalar1=bk_sb)

    if g < n_qgrp:
        psQ = pskq_pool.tile([128, 512], f32, tag="pskq", name="psQ")
        for j in range(n_j):
            nc.tensor.matmul(
                psQ, wq_sb[:, j, :], xtg[:, j, xs], start=(j == 0), stop=(j == n_j - 1)
            )
        nc.vector.tensor_scalar_add(out=QT_sb[:, c0:c1], in0=psQ, scalar1=bq_sb)

    psV = pskq_pool.tile([128, 512], f32, tag="pskq", name="psV")
    for j in range(n_j):
        nc.tensor.matmul(
            psV, wv_sb[:, j, :], xtg[:, j, xs], start=(j == 0), stop=(j == n_j - 1)
        )
    nc.vector.tensor_scalar_add(out=VT_st[:, c0:c1], in0=psV, scalar1=bv_sb)
    # un-transpose V on the TensorE (PE has slack in phase 1; keeps the
    # SP HWDGE queue free for the x transposes)
    for s in range(4):
        kt = g * 4 + s
        psT = psv_pool.tile([128, 128], bf16, tag="psT", name="psT")
        nc.tensor.transpose(psT, VT_st[:, kt * 128 : (kt + 1) * 128], ident)
        nc.vector.tensor_copy(V_all[:, kt, 0:dk], psT)


def _emit_pools(stack, tc, mybir, aps, n_j, n_kt, n_grp, n_qgrp, n_qb, dk):
    nc = tc.nc
    f32 = mybir.dt.float32
    bf16 = mybir.dt.bfloat16
    AF = mybir.ActivationFunctionType
    x, Wq, Wk, Wv, bq, bk, bv, out = aps
    scale = 1.0 / float(np.sqrt(dk))
    d_in = n_j * 128
    f_kv = n_kt * 128

    # ---- constants: weights (cast to bf16 during DMA), biases ----
    const_pool = stack.enter_context(tc.tile_pool(name="const", bufs=1))
    wq_sb = const_pool.tile([128, n_j, dk], bf16, name="wq_sb")
    wk_sb = const_pool.tile([128, n_j, dk], bf16, name="wk_sb")
    wv_sb = const_pool.tile([128, n_j, dk], bf16, name="wv_sb")
    # [p, j, m] = W[j*128 + p, m] so block j matches xT block j (d = j*128+p)
    nc.gpsimd.dma_start(out=wq_sb, in_=Wq.rearrange("(j p) m -> p j m", p=128))
    nc.gpsimd.dma_start(out=wk_sb, in_=Wk.rearrange("(j p) m -> p j m", p=128))
    nc.gpsimd.dma_start(out=wv_sb, in_=Wv.rearrange("(j p) m -> p j m", p=128))
    bq_sb = const_pool.tile([dk, 1], f32, name="bq_sb")
    bk_sb = const_pool.tile([dk, 1], f32, name="bk_sb")
    bv_sb = const_pool.tile([dk, 1], f32, name="bv_sb")
    nc.gpsimd.dma_start(out=bq_sb, in_=bq)
    nc.gpsimd.dma_start(out=bk_sb, in_=bk)
    nc.gpsimd.dma_start(out=bv_sb, in_=bv)

    # ---- persistent on-chip tensors ----
    big_pool = stack.enter_context(tc.tile_pool(name="big", bufs=1))
    KT_sb = big_pool.tile([128, f_kv], bf16, name="KT_sb")       # K^T  [d, k]
    QT_sb = big_pool.tile([128, n_qb * 512], bf16, name="QT_sb")  # Q^T  [d, q]
    VT_st = big_pool.tile([128, f_kv], bf16, name="VT_st")       # V^T  [dv, k]
    # row stride 144*2B = 288B: multiple of 32B so each xbar-transpose dst is aligned
    V_all = big_pool.tile([128, n_kt, 144], bf16, name="V_all")  # V|1  [k, kt, dv+1]

    # working pools (flat, no scoping -> no artificial barriers)
    xt_pool = stack.enter_context(tc.tile_pool(name="xt", bufs=3))
    e_pool = stack.enter_context(tc.tile_pool(name="eall", bufs=2))
    out_pool = stack.enter_context(tc.tile_pool(name="outp", bufs=3))
    pskq_pool = stack.enter_context(tc.tile_pool(name="pskq", bufs=2, space="PSUM"))
    psv_pool = stack.enter_context(tc.tile_pool(name="psv", bufs=1, space="PSUM"))
    psS_pool = stack.enter_context(tc.tile_pool(name="psS", bufs=2, space="PSUM"))
    psO_pool = stack.enter_context(tc.tile_pool(name="psO", bufs=1, space="PSUM"))

    # ones columns for the denominator (col dk of V_all); V transposes
    # overwrite cols 0:dk, cols dk:132 keep the 1.0 fill (only dk is read).
    nc.vector.memset(V_all, 1.0)

    # identity for TensorE transposes (V^T -> V)
    from concourse.masks import make_identity

    ident = const_pool.tile([128, 128], bf16, name="ident")
    make_identity(nc, ident)

    # ---- phase 1: transpose x, project K^T / Q^T / V^T, un-transpose V ----
    # SBUF-source xbar transposes are packet-bound (128-row source => 256B
    # writes per output partition). Instead: (1) SWDGE cast x fp32->bf16
    # HBM->HBM in large contiguous chunks, (2) xbar-transpose DRAM->SBUF with
    # sg_rows-row sources so each output partition gets sg_rows*2B contiguous.
    sg_rows = 1024 if f_kv % 1024 == 0 else 512
    g_per_sg = sg_rows // 512
    for sg in range(f_kv // sg_rows):
        r0, r1 = sg * sg_rows, (sg + 1) * sg_rows
        # one raw internal DRAM tensor per super-group: distinct tensors give
        # Tile precise per-sg dependencies (pool-level tracking is coarse)
        xbf = nc.dram_tensor(f"xbf{sg}", [sg_rows, d_in], bf16, kind="Internal").ap()
        nc.gpsimd.dma_start(out=xbf, in_=x[r0:r1, :])  # cast, HBM->HBM
        xtg = xt_pool.tile([128, n_j, sg_rows], bf16, tag="xt", name="xtg")
        for j in range(n_j):
            nc.sync.dma_start(
                out=xtg[:, j, :],
                in_=xbf[:, j * 128 : (j + 1) * 128],
                transpose=True,
            )
        for g in range(sg * g_per_sg, (sg + 1) * g_per_sg):
            _emit_proj_group(
                nc, mybir, g, xtg, (g % g_per_sg) * 512,
                n_j, n_qgrp, dk,
                (wq_sb, wk_sb, wv_sb, bq_sb, bk_sb, bv_sb, ident),
                (KT_sb, QT_sb, VT_st, V_all),
                (pskq_pool, psv_pool, out_pool),
            )

    # ---- phase 2: attention per query block of 512 ----
    # The attnV accumulation of block qb-1 is interleaved (in chunks of 8
    # matmuls) between the S^T matmul-pairs of block qb, so ACT (exp) and the
    # PE stream both stay busy instead of alternating.
    def attnv_mms(qb, Eall):
        for sub in range(4):
            for kt in range(n_kt):
                yield (qb, sub, kt, Eall)

    psO_live = {}

    def emit_attnv(item):
        qb, sub, kt, Eall = item
        if kt == 0:
            psO_live[sub] = psO_pool.tile([128, 132], f32, tag="psO", name="psO")
        nc.tensor.matmul(
            psO_live[sub][:, 0 : dk + 1],
            Eall[:, kt, sub * 128 : (sub + 1) * 128],
            V_all[:, kt, 0 : dk + 1],
            start=(kt == 0),
            stop=(kt == n_kt - 1),
        )
        if kt == n_kt - 1:
            psO = psO_live.pop(sub)
            recp = out_pool.tile([128, 1], f32, tag="recp", name="recp")
            nc.vector.reciprocal(recp, psO[:, dk : dk + 1])
            osb = out_pool.tile([128, dk], f32, tag="osb", name="osb")
            nc.vector.tensor_scalar_mul(osb, psO[:, 0:dk], recp)
            q0 = (qb * 4 + sub) * 128
            nc.sync.dma_start(out=out[q0 : q0 + 128, :], in_=osb)

    pending = None  # attnV generator for the previous q-block
    for qb in range(n_qb):
        Eall = e_pool.tile([128, n_kt, 512], bf16, tag="eall", name="Eall")
        for kth in range(n_kt // 2):
            psS = psS_pool.tile([128, 2, 512], f32, tag="psS", name="psS")
            for h in range(2):
                kt = kth * 2 + h
                nc.tensor.matmul(
                    psS[:, h, :],
                    KT_sb[:, kt * 128 : (kt + 1) * 128],
                    QT_sb[:, qb * 512 : (qb + 1) * 512],
                    start=True,
                    stop=True,
                )
            # exp over both k-tiles at once (FD=1024 amortizes ACT overhead)
            nc.scalar.activation(
                out=Eall[:, kth * 2 : kth * 2 + 2, :],
                in_=psS,
                func=AF.Exp,
                scale=scale,
            )
            if pending is not None:
                for _ in range(8):
                    item = next(pending, None)
                    if item is not None:
                        emit_attnv(item)
        if pending is not None:
            for item in pending:
                emit_attnv(item)
        pending = attnv_mms(qb, Eall)
    for item in pending:
        emit_attnv(item)


def build(f_kv=F, f_q=QH, d_in=D_IN, dk=DK, n_cores=N_CORES):
    """Build + bacc-compile the Bass module. Geometry must be multiples of 512."""
    key = (f_kv, f_q, d_in, dk, n_cores)
    if key in _CACHE:
        return _CACHE[key]
    import concourse.mybir as mybir
    import concourse.tile as tile
    from concourse import bacc

    f32 = mybir.dt.float32
    assert f_kv % 512 == 0 and f_q % 512 == 0 and d_in % 128 == 0

    nc = bacc.Bacc(
        "TRN2", target_bir_lowering=False, debug=False, num_devices=n_cores
    )
    x = nc.dram_tensor("x", [f_kv, d_in], f32, kind="ExternalInput").ap()
    Wq = nc.dram_tensor("Wq", [d_in, dk], f32, kind="ExternalInput").ap()
    Wk = nc.dram_tensor("Wk", [d_in, dk], f32, kind="ExternalInput").ap()
    Wv = nc.dram_tensor("Wv", [d_in, dk], f32, kind="ExternalInput").ap()
    bq = nc.dram_tensor("bq", [dk], f32, kind="ExternalInput").ap()
    bk = nc.dram_tensor("bk", [dk], f32, kind="ExternalInput").ap()
    bv = nc.dram_tensor("bv", [dk], f32, kind="ExternalInput").ap()
    out = nc.dram_tensor("out", [f_q, dk], f32, kind="ExternalOutput").ap()

    with tile.TileContext(nc) as tc:
        _emit(
            tc,
            (x, Wq, Wk, Wv, bq, bk, bv, out),
            n_j=d_in // 128,
            n_kt=f_kv // 128,
            n_grp=f_kv // 512,
            n_qgrp=f_q // 512,
            n_qb=f_q // 512,
            dk=dk,
        )
    nc.compile()
    _CACHE[key] = nc
    return nc


def _in_maps(x, Wq, bq, Wk, bk, Wv, bv):
    """Per-core inputs: batch c//2 with its query-half (c%2) rows first."""
    maps = []
    shared = {
        "Wq": np.ascontiguousarray(Wq, np.float32),
        "Wk": np.ascontiguousarray(Wk, np.float32),
        "Wv": np.ascontiguousarray(Wv, np.float32),
        "bq": np.ascontiguousarray(bq, np.float32),
        "bk": np.ascontiguousarray(bk, np.float32),
        "bv": np.ascontiguousarray(bv, np.float32),
    }
    for c in range(N_CORES):
        b, h = divmod(c, 2)
        xb = x[b]
        xperm = np.concatenate(
            [xb[h * QH : (h + 1) * QH], xb[(1 - h) * QH : (2 - h) * QH]], axis=0
        )
        maps.append({"x": np.ascontiguousarray(xperm, np.float32), **shared})
    return maps


def _ensure_ntff_hook():
    """Provide antenv.axon_hooks (absent in this image) so that
    run_bass_kernel_spmd(trace=True) can reach the libaxon NTFF profiler."""
    import sys
    import types

    if "antenv.axon_hooks" in sys.modules:
        return
    mod = types.ModuleType("antenv.axon_hooks")
    mod._hook = None
    mod.set_axon_ntff_profile_hook = lambda h: setattr(mod, "_hook", h)
    mod.get_axon_ntff_profile_hook = lambda: mod._hook
    sys.modules["antenv.axon_hooks"] = mod
    try:
        import antenv

        antenv.axon_hooks = mod
        from trn_agent_boot.trn_boot import _ntff_profile_via_ctypes

        mod._hook = _ntff_profile_via_ctypes("/opt/axon/libaxon_pjrt.so")
    except Exception:
        pass


def kernel(x, Wq, bq, Wk, bk, Wv, bv, trace=False, trace_cores=None):
    global LAST_RESULT
    _ensure_ntff_hook()
    from concourse import bass_utils

    nc = build()
    res = bass_utils.run_bass_kernel_spmd(
        nc,
        _in_maps(x, Wq, bq, Wk, bk, Wv, bv),
        core_ids=list(range(N_CORES)),
        trace=trace,
        trace_cores=trace_cores,
    )
    LAST_RESULT = res
    out = np.empty((B, F, DK), np.float32)
    for c, r in enumerate(res.results):
        b, h = divmod(c, 2)
        out[b, h * QH : (h + 1) * QH] = r["out"]
    return out


# revision 25
# speedup vs baseline: 1.4048x; 1.2710x over previous
"""Single-head attention (B=4, F=4096, D_IN=1024, D_K=D_V=128) on 8 Trainium2
NeuronCores via Bass/Tile.

Sharding: core c handles batch c//2 and query-half c%2 (2048 query rows), with
K/V computed over the full 4096-row batch on-core (K/V projection is cheap
relative to attention, so the pair-wise duplication is fine). The per-core x
input is row-permuted on host so that the core's query rows are rows 0..2047;
softmax over keys is permutation-invariant so key order does not matter.

On-core algorithm (bf16 matmuls, fp32 PSUM accumulation, softmax WITHOUT
max-subtraction -- scores are ~N(0, 0.33) for this problem's randn inputs, so
exp cannot overflow; verified in test.py):

  xT       = xbar-DMA-transpose(cast_bf16(x))     # [d_in, seq] in 128-blocks
  KT[d,k]  = sum_j Wk[j].T @ xT[j] + bk           # [128, 4096]  (d on partitions)
  QT[d,q]  = sum_j Wq[j].T @ xT[j] + bq           # [128, 2048]
  VT[d,k]  = sum_j Wv[j].T @ xT[j] + bv  --xbar-> V[k, dv] (+ ones column)
  per q-block of 512:
    ST[kt, q]   = KT[:,kt].T @ QT[:,qblock]       # PSUM, per k-tile
    E[kt, q]    = exp(ST * 1/sqrt(128))           # ACT, bf16, k on partitions
    O[q, 0:129] = sum_kt E[kt, qsub].T @ Vones[kt]  # numerator | denominator
    out[q, :]   = O[:, 0:128] * reciprocal(O[:, 128])
"""

import numpy as np

B, F, D_IN, DK = 4, 4096, 1024, 128
N_CORES = 8
QH = F // 2  # queries per core

_CACHE = {}
LAST_RESULT = None  # BassKernelResults of the most recent kernel() run


def _emit(tc, aps, n_j, n_kt, n_grp, n_qgrp, n_qb, dk):
    from contextlib import ExitStack

    import concourse.mybir as mybir

    with ExitStack() as stack:
        _emit_pools(stack, tc, mybir, aps, n_j, n_kt, n_grp, n_qgrp, n_qb, dk)


def _emit_proj_group(nc, mybir, g, xtg, xoff, n_j, n_qgrp, dk, consts, bigs, pools):
    """K^T/Q^T/V^T projections + PE un-transpose of V for one 512-seq group.

    xtg[:, j, xoff:xoff+512] holds x^T block j for this group's columns.
    """
    f32 = mybir.dt.float32
    bf16 = mybir.dt.bfloat16
    wq_sb, wk_sb, wv_sb, bq_sb, bk_sb, bv_sb, ident = consts
    KT_sb, QT_sb, VT_st, V_all = bigs
    pskq_pool, psv_pool, out_pool = pools

    c0, c1 = g * 512, (g + 1) * 512
    xs = slice(xoff, xoff + 512)
    psK = pskq_pool.tile([128, 512], f32, tag="pskq", name="psK")
    for j in range(n_j):
        nc.tensor.matmul(
            psK, wk_sb[:, j, :], xtg[:, j, xs], start=(j == 0), stop=(j == n_j - 1)
        )
    nc.vector.tensor_scalar_add(out=KT_sb[:, c0:c1], in0=psK, scalar1=bk_sb)

    if g < n_qgrp:
        psQ = pskq_pool.tile([128, 512], f32, tag="pskq", name="psQ")
        for j in range(n_j):
            nc.tensor.matmul(
                psQ, wq_sb[:, j, :], xtg[:, j, xs], start=(j == 0), stop=(j == n_j - 1)
            )
        nc.vector.tensor_scalar_add(out=QT_sb[:, c0:c1], in0=psQ, scalar1=bq_sb)

    psV = pskq_pool.tile([128, 512], f32, tag="pskq", name="psV")
    for j in range(n_j):
        nc.tensor.matmul(
            psV, wv_sb[:, j, :], xtg[:, j, xs], start=(j == 0), stop=(j == n_j - 1)
        )
    nc.vector.tensor_scalar_add(out=VT_st[:, c0:c1], in0=psV, scalar1=bv_sb)
    # un-transpose V on the TensorE (PE has slack in phase 1; keeps the
    # SP HWDGE queue free for the x transposes)
    for s in range(4):
        kt = g * 4 + s
        psT = psv_pool.tile([128, 128], bf16, tag="psT", name="psT")
        nc.tensor.transpose(psT, VT_st[:, kt * 128 : (kt + 1) * 128], ident)
        nc.vector.tensor_copy(V_all[:, kt, 0:dk], psT)


def _emit_pools(stack, tc, mybir, aps, n_j, n_kt, n_grp, n_qgrp, n_qb, dk):
    nc = tc.nc
    f32 = mybir.dt.float32
    bf16 = mybir.dt.bfloat16
    AF = mybir.ActivationFunctionType
    x, Wq, Wk, Wv, bq, bk, bv, out = aps
    scale = 1.0 / float(np.sqrt(dk))
    d_in = n_j * 128
    f_kv = n_kt * 128

    # ---- constants: weights (cast to bf16 during DMA), biases ----
    const_pool = stack.enter_context(tc.tile_pool(name="const", bufs=1))
    wq_sb = const_pool.tile([128, n_j, dk], bf16, name="wq_sb")
    wk_sb = const_pool.tile([128, n_j, dk], bf16, name="wk_sb")
    wv_sb = const_pool.tile([128, n_j, dk], bf16, name="wv_sb")
    # [p, j, m] = W[j*128 + p, m] so block j matches xT block j (d = j*128+p)
    nc.gpsimd.dma_start(out=wq_sb, in_=Wq.rearrange("(j p) m -> p j m", p=128))
    nc.gpsimd.dma_start(out=wk_sb, in_=Wk.rearrange("(j p) m -> p j m", p=128))
    nc.gpsimd.dma_start(out=wv_sb, in_=Wv.rearrange("(j p) m -> p j m", p=128))
    bq_sb = const_pool.tile([dk, 1], f32, name="bq_sb")
    bk_sb = const_pool.tile([dk, 1], f32, name="bk_sb")
    bv_sb = const_pool.tile([dk, 1], f32, name="bv_sb")
    nc.gpsimd.dma_start(out=bq_sb, in_=bq)
    nc.gpsimd.dma_start(out=bk_sb, in_=bk)
    nc.gpsimd.dma_start(out=bv_sb, in_=bv)

    # ---- persistent on-chip tensors ----
    big_pool = stack.enter_context(tc.tile_pool(name="big", bufs=1))
    KT_sb = big_pool.tile([128, f_kv], bf16, name="KT_sb")       # K^T  [d, k]
    QT_sb = big_pool.tile([128, n_qb * 512], bf16, name="QT_sb")  # Q^T  [d, q]
    VT_st = big_pool.tile([128, f_kv], bf16, name="VT_st")       # V^T  [dv, k]
    # row stride 144*2B = 288B: multiple of 32B so each xbar-transpose dst is aligned
    V_all = big_pool.tile([128, n_kt, 144], bf16, name="V_all")  # V|1  [k, kt, dv+1]

    # working pools (flat, no scoping -> no artificial barriers)
    xt_pool = stack.enter_context(tc.tile_pool(name="xt", bufs=2))
    e_pool = stack.enter_context(tc.tile_pool(name="eall", bufs=3))
    out_pool = stack.enter_context(tc.tile_pool(name="outp", bufs=3))
    pskq_pool = stack.enter_context(tc.tile_pool(name="pskq", bufs=2, space="PSUM"))
    psv_pool = stack.enter_context(tc.tile_pool(name="psv", bufs=1, space="PSUM"))
    psS_pool = stack.enter_context(tc.tile_pool(name="psS", bufs=2, space="PSUM"))
    psO_pool = stack.enter_context(tc.tile_pool(name="psO", bufs=1, space="PSUM"))

    # ones columns for the denominator (col dk of V_all); V transposes
    # overwrite cols 0:dk, cols dk:132 keep the 1.0 fill (only dk is read).
    nc.vector.memset(V_all, 1.0)

    # identity for TensorE transposes (V^T -> V)
    from concourse.masks import make_identity

    ident = const_pool.tile([128, 128], bf16, name="ident")
    make_identity(nc, ident)

    # ---- phase 1: transpose x, project K^T / Q^T / V^T, un-transpose V ----
    # SBUF-source xbar transposes are packet-bound (128-row source => 256B
    # writes per output partition). Instead: (1) SWDGE cast x fp32->bf16
    # HBM->HBM in large contiguous chunks, (2) xbar-transpose DRAM->SBUF with
    # sg_rows-row sources so each output partition gets sg_rows*2B contiguous.
    sg_rows = 1024 if f_kv % 1024 == 0 else 512
    g_per_sg = sg_rows // 512
    dram_pool = stack.enter_context(tc.tile_pool(name="dram", bufs=1, space="DRAM"))

    # ---- attention emission helpers ----
    def attnv_mms(qb, Eall):
        for sub in range(4):
            for kt in range(n_kt):
                yield (qb, sub, kt, Eall)

    psO_live = {}

    def emit_attnv(item):
        qb, sub, kt, Eall = item
        if kt == 0:
            psO_live[sub] = psO_pool.tile([128, 132], f32, tag="psO", name="psO")
        nc.tensor.matmul(
            psO_live[sub][:, 0 : dk + 1],
            Eall[:, kt, sub * 128 : (sub + 1) * 128],
            V_all[:, kt, 0 : dk + 1],
            start=(kt == 0),
            stop=(kt == n_kt - 1),
        )
        if kt == n_kt - 1:
            psO = psO_live.pop(sub)
            recp = out_pool.tile([128, 1], f32, tag="recp", name="recp")
            nc.vector.reciprocal(recp, psO[:, dk : dk + 1])
            osb = out_pool.tile([128, dk], f32, tag="osb", name="osb")
            nc.vector.tensor_scalar_mul(osb, psO[:, 0:dk], recp)
            q0 = (qb * 4 + sub) * 128
            nc.sync.dma_start(out=out[q0 : q0 + 128, :], in_=osb)

    Ealls = {}
    st_done = {qb: 0 for qb in range(n_qb)}  # kth pairs emitted per q-block

    def emit_st_pair(qb, kth):
        if qb not in Ealls:
            Ealls[qb] = e_pool.tile([128, n_kt, 512], bf16, tag="eall", name="Eall")
        Eall = Ealls[qb]
        psS = psS_pool.tile([128, 2, 512], f32, tag="psS", name="psS")
        for h in range(2):
            kt = kth * 2 + h
            nc.tensor.matmul(
                psS[:, h, :],
                KT_sb[:, kt * 128 : (kt + 1) * 128],
                QT_sb[:, qb * 512 : (qb + 1) * 512],
                start=True,
                stop=True,
            )
        # exp over both k-tiles at once (FD=1024 amortizes ACT overhead)
        nc.scalar.activation(
            out=Eall[:, kth * 2 : kth * 2 + 2, :], in_=psS, func=AF.Exp, scale=scale
        )
        st_done[qb] += 1

    # ---- phase 1: stage/transpose x, projections; S^T+exp of the first two
    # q-blocks are interleaved as soon as their K^T/Q^T columns exist, so the
    # ACT exp stream (the phase-2 critical path) starts ~50us earlier ----
    for sg in range(f_kv // sg_rows):
        r0, r1 = sg * sg_rows, (sg + 1) * sg_rows
        # one DRAM staging tile per super-group: keeps Tile's dependency
        # tracking per-sg so transposes of sg start as soon as ITS cast lands
        xbf = dram_pool.tile([sg_rows, d_in], bf16, tag=f"xbf{sg}", name="xbf")
        nc.gpsimd.dma_start(out=xbf, in_=x[r0:r1, :])  # cast, HBM->HBM
        xtg = xt_pool.tile([128, n_j, sg_rows], bf16, tag="xt", name="xtg")
        for j in range(n_j):
            nc.sync.dma_start(
                out=xtg[:, j, :],
                in_=xbf[:, j * 128 : (j + 1) * 128],
                transpose=True,
            )
        for g in range(sg * g_per_sg, (sg + 1) * g_per_sg):
            _emit_proj_group(
                nc, mybir, g, xtg, (g % g_per_sg) * 512,
                n_j, n_qgrp, dk,
                (wq_sb, wk_sb, wv_sb, bq_sb, bk_sb, bv_sb, ident),
                (KT_sb, QT_sb, VT_st, V_all),
                (pskq_pool, psv_pool, out_pool),
            )
            ready_kth = min(2 * (g + 1), n_kt // 2)
            for qb in range(min(2, n_qb)):
                if qb <= g:
                    while st_done[qb] < ready_kth:
                        emit_st_pair(qb, st_done[qb])

    # ---- phase 2: remaining S^T blocks; attnV of block qb-2 interleaved (in
    # chunks of 8 matmuls) between S^T pairs so PE and ACT both stream ----
    pending = [attnv_mms(qb, Ealls[qb]) for qb in range(min(2, n_qb))]
    for qb in range(2, n_qb):
        while st_done[qb] < n_kt // 2:
            emit_st_pair(qb, st_done[qb])
            if pending:
                for _ in range(8):
                    item = next(pending[0], None)
                    if item is None:
                        pending.pop(0)
                        break
                    emit_attnv(item)
        pending.append(attnv_mms(qb, Ealls[qb]))
    for gen in pending:
        for item in gen:
            emit_attnv(item)


def build(f_kv=F, f_q=QH, d_in=D_IN, dk=DK, n_cores=N_CORES):
    """Build + bacc-compile the Bass module. Geometry must be multiples of 512."""
    key = (f_kv, f_q, d_in, dk, n_cores)
    if key in _CACHE:
        return _CACHE[key]
    import concourse.mybir as mybir
    import concourse.tile as tile
    from concourse import bacc

    f32 = mybir.dt.float32
    assert f_kv % 512 == 0 and f_q % 512 == 0 and d_in % 128 == 0

    nc = bacc.Bacc(
        "TRN2", target_bir_lowering=False, debug=False, num_devices=n_cores
    )
    x = nc.dram_tensor("x", [f_kv, d_in], f32, kind="ExternalInput").ap()
    Wq = nc.dram_tensor("Wq", [d_in, dk], f32, kind="ExternalInput").ap()
    Wk = nc.dram_tensor("Wk", [d_in, dk], f32, kind="ExternalInput").ap()
    Wv = nc.dram_tensor("Wv", [d_in, dk], f32, kind="ExternalInput").ap()
    bq = nc.dram_tensor("bq", [dk], f32, kind="ExternalInput").ap()
    bk = nc.dram_tensor("bk", [dk], f32, kind="ExternalInput").ap()
    bv = nc.dram_tensor("bv", [dk], f32, kind="ExternalInput").ap()
    out = nc.dram_tensor("out", [f_q, dk], f32, kind="ExternalOutput").ap()

    with tile.TileContext(nc) as tc:
        _emit(
            tc,
            (x, Wq, Wk, Wv, bq, bk, bv, out),
            n_j=d_in // 128,
            n_kt=f_kv // 128,
            n_grp=f_kv // 512,
            n_qgrp=f_q // 512,
            n_qb=f_q // 512,
            dk=dk,
        )
    nc.compile()
    _CACHE[key] = nc
    return nc


def _in_maps(x, Wq, bq, Wk, bk, Wv, bv):
    """Per-core inputs: batch c//2 with its query-half (c%2) rows first."""
    maps = []
    shared = {
        "Wq": np.ascontiguousarray(Wq, np.float32),
        "Wk": np.ascontiguousarray(Wk, np.float32),
        "Wv": np.ascontiguousarray(Wv, np.float32),
        "bq": np.ascontiguousarray(bq, np.float32),
        "bk": np.ascontiguousarray(bk, np.float32),
        "bv": np.ascontiguousarray(bv, np.float32),
    }
    for c in range(N_CORES):
        b, h = divmod(c, 2)
        xb = x[b]
        xperm = np.concatenate(
            [xb[h * QH : (h + 1) * QH], xb[(1 - h) * QH : (2 - h) * QH]], axis=0
        )
        maps.append({"x": np.ascontiguousarray(xperm, np.float32), **shared})
    return maps


def _ensure_ntff_hook():
    """Provide antenv.axon_hooks (absent in this image) so that
    run_bass_kernel_spmd(trace=True) can reach the libaxon NTFF profiler."""
    import sys
    import types

    if "antenv.axon_hooks" in sys.modules:
        return
    mod = types.ModuleType("antenv.axon_hooks")
    mod._hook = None
    mod.set_axon_ntff_profile_hook = lambda h: setattr(mod, "_hook", h)
    mod.get_axon_ntff_profile_hook = lambda: mod._hook
    sys.modules["antenv.axon_hooks"] = mod
    try:
        import antenv

        antenv.axon_hooks = mod
        from trn_agent_boot.trn_boot import _ntff_profile_via_ctypes

        mod._hook = _ntff_profile_via_ctypes("/opt/axon/libaxon_pjrt.so")
    except Exception:
        pass


def kernel(x, Wq, bq, Wk, bk, Wv, bv, trace=False, trace_cores=None):
    global LAST_RESULT
    _ensure_ntff_hook()
    from concourse import bass_utils

    nc = build()
    res = bass_utils.run_bass_kernel_spmd(
        nc,
        _in_maps(x, Wq, bq, Wk, bk, Wv, bv),
        core_ids=list(range(N_CORES)),
        trace=trace,
        trace_cores=trace_cores,
    )
    LAST_RESULT = res
    out = np.empty((B, F, DK), np.float32)
    for c, r in enumerate(res.results):
        b, h = divmod(c, 2)
        out[b, h * QH : (h + 1) * QH] = r["out"]
    return out
